# revision 32
# baseline (speedup 1.0000x reference)
"""DeepFilter kernel for Trainium2 (8 NeuronCores, batch-parallel).

Math: the reference shifts input and filter by the SAME (df, dt) tap offset,
so the op factorizes into pointwise products followed by a separable 3x5
zero-padded box sum:
    P_r = ir*fr - ii*fi ; P_i = 2*ir*fi
    out_r = boxsum_3x5(P_r) ; out_i = boxsum_3x5(P_i)
    out = concat([out_r, out_i], axis=1)            # [B, 2F, T]

Per-core layout: F on partitions (2 full chunks + merged 12-row tail),
T on the free dim (TH-col pieces + 2-col halo).  The whole elementwise
chain runs on DVE as scalar_tensor_tensor ops (measured ~2x faster than
tensor_tensor, and keeping one engine means zero cross-engine semaphores
inside a piece); TensorE applies the F-box (banded matmul) accumulating
in PSUM; ScalarE copies PSUM->SBUF staging.

DMA queue layout (the big lever on this part): HBM->SBUF loads all go
through the SP HWDGE ring, which stripes a 128-partition transfer across
all 16 SDMA engines; SBUF->HBM stores go through the GPSIMD SWDGE ring.
NTFF profiles show HBM reads starve writes at the arbitration level:
store descriptors queue up but store bytes stop flowing whenever the
read queues are full, so execution alternates read phases (~410 GB/s)
and write phases (~165-210 GB/s) and the total is near their serialized
sum.  Stores on an HWDGE ring are far worse still (single-engine
~21 GB/s drain while reads are active, back-pressuring the pipeline),
so stores must live on the SWDGE ring.  Issuing loads from an engine
with no other work (SP) avoids head-of-line blocking -- an ACT-ring
load issue waiting on a tile-recycle semaphore stalls the PSUM-drain
copies queued behind it.
"""

import numpy as np

B, F, T = 16, 257, 4000
NCORES = 8
B_LOC = B // NCORES  # 2
P = 128
NT = 500  # psum tile width (<=512 fp32 matmul moving-operand limit)

# Regular F chunks: (first loaded row, n rows loaded,
#                    valid psum partitions [lo,hi), first output f row)
#  c0: rows 0..127   -> f 0..126  at partitions 0..126
#  c1: rows 126..253 -> f 127..252 at partitions 1..126
# The tail (f 253..256) is handled by a merged macro-tile covering BOTH
# batches: partitions b*6+r hold rows 251+r of batch b; a block-diagonal
# [12,8] band produces f 253..256 for b0 at partitions 0..3, b1 at 4..7.
CHUNKS = [
    (0, 128, 0, 127, 0),
    (126, 128, 1, 127, 127),
]
C2_FL0, C2_NROWS_B, C2_FO0, C2_NF = 251, 6, 253, 4

DEFAULT_TH = 1000
DEFAULT_BUFS = dict(inp=16, prod=8, pair=8, stg=8, ps=8)
# engine assignment: loads (ir, ii, fr, fi), stores (r, i)
DEFAULT_CFG = dict(load_eng=("sync", "sync", "sync", "sync"),
                   store_eng=("gpsimd", "gpsimd"), store_per_j=False,
                   elem="stt", edge_split=False, const_eng="scalar")

_CACHE = {}


def _band_matrices():
    k = np.arange(P)
    band = (np.abs(k[:, None] - k[None, :]) <= 1).astype(np.float32)
    w6 = np.zeros((12, 8), np.float32)
    for bb in range(2):
        for r in range(6):
            for m in range(4):
                if abs(m + 2 - r) <= 1:
                    w6[bb * 6 + r, bb * 4 + m] = 1.0
    return band, w6


def _band_matrices_all():
    import ml_dtypes
    band, w6 = _band_matrices()
    return {"wp": band, "w6p": w6,
            "wpb": band.astype(ml_dtypes.bfloat16),
            "w6pb": w6.astype(ml_dtypes.bfloat16)}


def _build_module(repeats=1, th=DEFAULT_TH, bufs=None, dma_only=False,
                  cfg=None):
    import concourse.bacc as bacc
    import concourse.mybir as mybir
    import concourse.tile as tile

    cfg = dict(DEFAULT_CFG, **(cfg or {}))
    bufs = dict(DEFAULT_BUFS, **(bufs or {}))
    assert T % th == 0 and th % NT == 0
    n_pieces = T // th
    nj = th // NT
    tw = th + 4

    f32 = mybir.dt.float32
    f32r = mybir.dt.float32r
    mult = mybir.AluOpType.mult

    nc = bacc.Bacc("TRN2", target_bir_lowering=False, debug=False,
                   num_devices=NCORES)

    ins = {
        name: nc.dram_tensor(name, [B_LOC, F, T], f32, kind="ExternalInput")
        for name in ("inputs_r", "inputs_i", "filters_r", "filters_i")
    }
    bf16 = mybir.dt.bfloat16
    wp_d = nc.dram_tensor("wp", [P, P], f32r, kind="ExternalInput")
    w6p_d = nc.dram_tensor("w6p", [12, 8], f32r, kind="ExternalInput")
    wpb_d = nc.dram_tensor("wpb", [P, P], bf16, kind="ExternalInput")
    w6pb_d = nc.dram_tensor("w6pb", [12, 8], bf16, kind="ExternalInput")
    out_d = nc.dram_tensor("out", [B_LOC, 2 * F, T], f32, kind="ExternalOutput")

    ir_ap, ii_ap, fr_ap, fi_ap = (ins[n].ap() for n in
                                  ("inputs_r", "inputs_i", "filters_r",
                                   "filters_i"))
    out_ap = out_d.ap()

    with tile.TileContext(nc) as tc:
        with (
            tc.tile_pool(name="const", bufs=1) as cpool,
            tc.tile_pool(name="inp", bufs=bufs["inp"]) as ipool,
            tc.tile_pool(name="prod", bufs=bufs["prod"]) as rpool,
            tc.tile_pool(name="pair", bufs=bufs["pair"]) as wpool,
            tc.tile_pool(name="stg", bufs=bufs["stg"]) as spool,
            tc.tile_pool(name="ps", bufs=bufs["ps"], space="PSUM") as qpool,
        ):
            wp_s = cpool.tile([P, P], f32r, name="wp_s", tag="wp_s")
            w6p_s = cpool.tile([12, 8], f32r, name="w6p_s", tag="w6p_s")
            wpb_s = cpool.tile([P, P], bf16, name="wpb_s", tag="wpb_s")
            w6pb_s = cpool.tile([12, 8], bf16, name="w6pb_s", tag="w6pb_s")
            # const loads go out the ACT ring (idle at t=0) so they don't
            # sit ahead of piece-0's input loads in the SP ring
            ce = getattr(nc, cfg.get("const_eng", "sync"))
            ce.dma_start(out=wp_s[:, :], in_=wp_d.ap()[:, :])
            ce.dma_start(out=w6p_s[:, :], in_=w6p_d.ap()[:, :])
            ce.dma_start(out=wpb_s[:, :], in_=wpb_d.ap()[:, :])
            ce.dma_start(out=w6pb_s[:, :], in_=w6pb_d.ap()[:, :])

            def emit_piece(t0, thp, loads, nrows, wpL, wpLb, np_out, vp1,
                           stores):
                """One macro-tile: T cols [t0, t0+thp), given per-batch
                loads [(part_off, b, fl0, nr)], band slices, valid psum
                rows [0,vp1), stores [(stage p0, p1, b, first f row)]."""
                tw_p = thp + 4
                nj_p = thp // NT
                # tile col c <-> t = t0 - 2 + c ; clip to [0, T)
                c_lo = max(0, 2 - t0)
                c_hi = tw_p - max(0, t0 + thp + 2 - T)
                t_lo, t_hi = t0 - 2 + c_lo, t0 - 2 + c_hi

                ir_t = ipool.tile([P, tw_p], f32, name="ir_t", tag="inp")
                ii_t = ipool.tile([P, tw_p], f32, name="ii_t", tag="inp")
                fr_t = ipool.tile([P, tw_p], f32, name="fr_t", tag="inp")
                fi_t = ipool.tile([P, tw_p], f32, name="fi_t", tag="inp")
                # loads split across DMA issue paths per cfg so transfers
                # from different tensors can overlap instead of sitting
                # FIFO in one ring
                le = [getattr(nc, e) for e in cfg["load_eng"]]
                # ir, fr first: the chain's first op t1 = ir*fr can
                # start as soon as two transfers land
                for t_sb, src, eng in ((ir_t, ir_ap, le[0]),
                                       (fr_t, fr_ap, le[2]),
                                       (ii_t, ii_ap, le[1]),
                                       (fi_t, fi_ap, le[3])):
                    for p_off, b, fl0, nr in loads:
                        eng.dma_start(
                            out=t_sb[p_off:p_off + nr, c_lo:c_hi],
                            in_=src[b, fl0:fl0 + nr, t_lo:t_hi])
                    # zero halo cols at the global T edges so the products
                    # are zero there (zero-pad semantics) and matmuls can
                    # always run full-width (fp32r needs even widths)
                    if c_lo > 0:
                        nc.vector.memset(t_sb[0:nrows, 0:c_lo], 0.0)
                    if c_hi < tw_p:
                        nc.vector.memset(t_sb[0:nrows, c_hi:tw_p], 0.0)

                if dma_only:
                    # measurement variant: identical DMA traffic, no
                    # compute -- stores forward slices of the loads
                    de_r, de_i = (getattr(nc, e) for e in cfg["store_eng"])
                    for sp0, sp1, b, fo0 in stores:
                        n_f = sp1 - sp0
                        de_r.dma_start(
                            out=out_ap[b, fo0:fo0 + n_f, t0:t0 + thp],
                            in_=ir_t[sp0:sp1, 2:2 + thp])
                        de_i.dma_start(
                            out=out_ap[b, F + fo0:F + fo0 + n_f, t0:t0 + thp],
                            in_=ii_t[sp0:sp1, 2:2 + thp])
                    return

                elem = cfg["elem"]
                add = mybir.AluOpType.add
                sub = mybir.AluOpType.subtract
                nr = nrows

                def stt(out, a, scl, b, op1):
                    # scalar_tensor_tensor runs ~2x faster than
                    # tensor_tensor on DVE (measured): express every
                    # elementwise op as (in0 op0 scalar) op1 in1
                    nc.vector.scalar_tensor_tensor(
                        out=out, in0=a, scalar=scl, in1=b,
                        op0=mult, op1=op1)

                if elem == "pe_rij5":
                    # 3 DVE products in bf16; the whole 5-tap T-box and
                    # the r-plane subtraction run on PE: 15 accumulating
                    # matmuls per psum pair, all sharing one bf16 band
                    # (the minus is folded into t2 = -(ii*fi)).
                    t1_t = rpool.tile([P, tw_p], bf16, name="t1_t",
                                      tag="prod")
                    t2_t = rpool.tile([P, tw_p], bf16, name="t2_t",
                                      tag="prod")
                    pi_t = rpool.tile([P, tw_p], bf16, name="pi_t",
                                      tag="prod")
                    stt(t1_t[0:nr, 0:tw_p], ir_t[0:nr, 0:tw_p],
                        1.0, fr_t[0:nr, 0:tw_p], mult)
                    stt(t2_t[0:nr, 0:tw_p], ii_t[0:nr, 0:tw_p],
                        -1.0, fi_t[0:nr, 0:tw_p], mult)
                    stt(pi_t[0:nr, 0:tw_p], ir_t[0:nr, 0:tw_p],
                        2.0, fi_t[0:nr, 0:tw_p], mult)

                    def groups_for(j, ps_r, ps_i):
                        return (
                            (ps_i,
                             [(pi_t, wpLb, NT * j + s) for s in range(5)]),
                            (ps_r,
                             [(t1_t, wpLb, NT * j + s) for s in range(5)]
                             + [(t2_t, wpLb, NT * j + s)
                                for s in range(5)]),
                        )
                elif elem == "pe_i5":
                    # i-plane skips its pair sum: 5 direct bf16 matmuls
                    # on pi; r-plane keeps the fp32r pair-sum scheme
                    t1_t = rpool.tile([P, tw_p], f32r, name="t1_t",
                                      tag="prod")
                    t2_t = rpool.tile([P, tw_p], f32r, name="t2_t",
                                      tag="prod")
                    pi_t = rpool.tile([P, tw_p], bf16, name="pi_t",
                                      tag="prod")
                    pr_t = t1_t
                    qr_t = wpool.tile([P, tw_p], f32r, name="qr_t",
                                      tag="pair")
                    stt(t1_t[0:nr, 0:tw_p], ir_t[0:nr, 0:tw_p],
                        1.0, fr_t[0:nr, 0:tw_p], mult)
                    stt(t2_t[0:nr, 0:tw_p], ii_t[0:nr, 0:tw_p],
                        1.0, fi_t[0:nr, 0:tw_p], mult)
                    stt(pi_t[0:nr, 0:tw_p], ir_t[0:nr, 0:tw_p],
                        2.0, fi_t[0:nr, 0:tw_p], mult)
                    stt(pr_t[0:nr, 0:tw_p], t1_t[0:nr, 0:tw_p],
                        1.0, t2_t[0:nr, 0:tw_p], sub)
                    stt(qr_t[0:nr, 0:tw_p - 1],
                        pr_t[0:nr, 0:tw_p - 1], 1.0, pr_t[0:nr, 1:tw_p], add)

                    def groups_for(j, ps_r, ps_i):
                        return (
                            (ps_i,
                             [(pi_t, wpLb, NT * j + s) for s in range(5)]),
                            (ps_r,
                             [(qr_t, wpL, NT * j),
                              (qr_t, wpL, NT * j + 2),
                              (pr_t, wpL, NT * j + 4)]),
                        )
                else:
                    # float32r: PE matmuls on fp32r run 4x faster than
                    # fp32; DVE rounds the products on write.  pr = t1-t2
                    # on DVE means both planes share ONE wp band: 6
                    # matmuls per psum pair.
                    t1_t = rpool.tile([P, tw_p], f32r, name="t1_t",
                                      tag="prod")
                    t2_t = rpool.tile([P, tw_p], f32r, name="t2_t",
                                      tag="prod")
                    pi_t = rpool.tile([P, tw_p], f32r, name="pi_t",
                                      tag="prod")
                    pr_t = t1_t
                    qr_t = wpool.tile([P, tw_p], f32r, name="qr_t",
                                      tag="pair")
                    qi_t = wpool.tile([P, tw_p], f32r, name="qi_t",
                                      tag="pair")
                    if elem.startswith("stt"):
                        stt(t1_t[0:nr, 0:tw_p], ir_t[0:nr, 0:tw_p],
                            1.0, fr_t[0:nr, 0:tw_p], mult)
                        stt(t2_t[0:nr, 0:tw_p], ii_t[0:nr, 0:tw_p],
                            1.0, fi_t[0:nr, 0:tw_p], mult)
                        stt(pi_t[0:nr, 0:tw_p], ir_t[0:nr, 0:tw_p],
                            2.0, fi_t[0:nr, 0:tw_p], mult)
                        # gpsimd cannot run STT (TensorScalarPtr is not
                        # a Pool-engine op on V3) -- keep the chain on
                        # DVE: in-order issue, zero cross-engine sems
                        stt(pr_t[0:nr, 0:tw_p], t1_t[0:nr, 0:tw_p],
                            1.0, t2_t[0:nr, 0:tw_p], sub)
                        if elem == "stt_qr_gps":
                            nc.gpsimd.tensor_add(qr_t[0:nr, 0:tw_p - 1],
                                                 pr_t[0:nr, 0:tw_p - 1],
                                                 pr_t[0:nr, 1:tw_p])
                        else:
                            stt(qr_t[0:nr, 0:tw_p - 1],
                                pr_t[0:nr, 0:tw_p - 1], 1.0,
                                pr_t[0:nr, 1:tw_p], add)
                        stt(qi_t[0:nr, 0:tw_p - 1],
                            pi_t[0:nr, 0:tw_p - 1], 1.0, pi_t[0:nr, 1:tw_p],
                            add)
                    else:
                        nc.vector.tensor_mul(t1_t[0:nr, 0:tw_p],
                                             ir_t[0:nr, 0:tw_p],
                                             fr_t[0:nr, 0:tw_p])
                        nc.vector.tensor_mul(t2_t[0:nr, 0:tw_p],
                                             ii_t[0:nr, 0:tw_p],
                                             fi_t[0:nr, 0:tw_p])
                        nc.vector.scalar_tensor_tensor(
                            out=pi_t[0:nr, 0:tw_p],
                            in0=ir_t[0:nr, 0:tw_p], scalar=2.0,
                            in1=fi_t[0:nr, 0:tw_p], op0=mult, op1=mult)
                        nc.gpsimd.tensor_sub(pr_t[0:nr, 0:tw_p],
                                             t1_t[0:nr, 0:tw_p],
                                             t2_t[0:nr, 0:tw_p])
                        nc.gpsimd.tensor_add(qr_t[0:nr, 0:tw_p - 1],
                                             pr_t[0:nr, 0:tw_p - 1],
                                             pr_t[0:nr, 1:tw_p])
                        nc.vector.tensor_add(qi_t[0:nr, 0:tw_p - 1],
                                             pi_t[0:nr, 0:tw_p - 1],
                                             pi_t[0:nr, 1:tw_p])

                    def groups_for(j, ps_r, ps_i):
                        return (
                            (ps_i,
                             [(qi_t, wpL, NT * j),
                              (qi_t, wpL, NT * j + 2),
                              (pi_t, wpL, NT * j + 4)]),
                            (ps_r,
                             [(qr_t, wpL, NT * j),
                              (qr_t, wpL, NT * j + 2),
                              (pr_t, wpL, NT * j + 4)]),
                        )

                stg_r = spool.tile([P, thp], f32, name="stg_r", tag="stg")
                stg_i = spool.tile([P, thp], f32, name="stg_i", tag="stg")
                se_r, se_i = (getattr(nc, e) for e in cfg["store_eng"])

                for j in range(nj_p):
                    ps_r = qpool.tile([P, NT], f32, name="ps_r", tag="ps")
                    ps_i = qpool.tile([P, NT], f32, name="ps_i", tag="ps")
                    for ps, mms in groups_for(j, ps_r, ps_i):
                        for k, (plane, wL, c_start) in enumerate(mms):
                            nc.tensor.matmul(
                                ps[0:np_out, 0:NT],
                                wL,
                                plane[0:nrows, c_start:c_start + NT],
                                start=(k == 0),
                                stop=(k == len(mms) - 1))
                    # PSUM reads must start at partition 0: copy rows
                    # 0:vp1 and let the store DMAs pick their slices.
                    nc.scalar.copy(
                        out=stg_r[0:vp1, NT * j:NT * (j + 1)],
                        in_=ps_r[0:vp1, 0:NT])
                    nc.scalar.copy(
                        out=stg_i[0:vp1, NT * j:NT * (j + 1)],
                        in_=ps_i[0:vp1, 0:NT])
                    if cfg["store_per_j"]:
                        for sp0, sp1, b, fo0 in stores:
                            n_f = sp1 - sp0
                            t_j0 = t0 + NT * j
                            se_r.dma_start(
                                out=out_ap[b, fo0:fo0 + n_f,
                                           t_j0:t_j0 + NT],
                                in_=stg_r[sp0:sp1, NT * j:NT * (j + 1)])
                            se_i.dma_start(
                                out=out_ap[b, F + fo0:F + fo0 + n_f,
                                           t_j0:t_j0 + NT],
                                in_=stg_i[sp0:sp1, NT * j:NT * (j + 1)])

                if not cfg["store_per_j"]:
                    for sp0, sp1, b, fo0 in stores:
                        n_f = sp1 - sp0
                        se_r.dma_start(
                            out=out_ap[b, fo0:fo0 + n_f, t0:t0 + thp],
                            in_=stg_r[sp0:sp1, 0:thp])
                        se_i.dma_start(
                            out=out_ap[b, F + fo0:F + fo0 + n_f, t0:t0 + thp],
                            in_=stg_i[sp0:sp1, 0:thp])

            def chunk_schedule(split_head, split_tail):
                """(t0, thp) pieces covering [0, T); optionally split the
                first/last th-piece into NT-wide slivers to shorten the
                stream's pipeline fill/drain latency."""
                s = [(th * h, th) for h in range(n_pieces)]
                out = []
                for i, (t0, thp) in enumerate(s):
                    if (split_head and i == 0) or (split_tail and
                                                   i == n_pieces - 1):
                        out += [(t0 + k * NT, NT) for k in range(thp // NT)]
                    else:
                        out.append((t0, thp))
                return out

            # piece sequence: regular (b, chunk, h) pieces, with the
            # DMA-light merged-tail pieces interleaved mid-stream
            edge = cfg.get("edge_split", False)
            for _rep in range(repeats):
                half = (n_pieces + 1) // 2
                n_ch = len(CHUNKS)
                for b in range(B_LOC):
                    for ci, (fl0, nrows, vp0, vp1, fo0) in enumerate(CHUNKS):
                        first = edge and b == 0 and ci == 0
                        last = edge and b == B_LOC - 1 and ci == n_ch - 1
                        for t0, thp in chunk_schedule(first, last):
                            emit_piece(
                                t0, thp, [(0, b, fl0, nrows)], nrows,
                                wp_s[:, :], wpb_s[:, :], P, vp1,
                                [(vp0, vp1, b, fo0)])
                    # merged tail pieces: first half after batch 0,
                    # second half after batch 1
                    hs = range(0, half) if b == 0 else range(half, n_pieces)
                    for h in hs:
                        emit_piece(
                            th * h, th,
                            [(0, 0, C2_FL0, C2_NROWS_B),
                             (6, 1, C2_FL0, C2_NROWS_B)],
                            12, w6p_s[0:12, 0:8], w6pb_s[0:12, 0:8], 8, 8,
                            [(0, 4, 0, C2_FO0), (4, 8, 1, C2_FO0)])

    nc.compile()
    return nc


def _get_module(repeats=1, th=DEFAULT_TH, bufs=None, dma_only=False,
                cfg=None):
    key = (f"nc{repeats}_{th}_{sorted((bufs or {}).items())}_{dma_only}"
           f"_{sorted((cfg or {}).items())}")
    if key not in _CACHE:
        _CACHE[key] = _build_module(repeats, th, bufs, dma_only, cfg)
    return _CACHE[key]


def _runner():
    """Build (once) a reusable jitted 8-core runner for the module."""
    if "runner" in _CACHE:
        return _CACHE["runner"]
    import jax
    import concourse.mybir as mybir
    from concourse import bass2jax
    from jax.sharding import Mesh, NamedSharding, PartitionSpec
    from jax.experimental.shard_map import shard_map

    nc = _get_module()
    bass2jax.install_neuronx_cc_hook()

    partition_name = (nc.partition_id_tensor.name
                      if nc.partition_id_tensor else None)
    in_names, out_names, out_avals, zero_outs = [], [], [], []
    for alloc in nc.m.functions[0].allocations:
        if not isinstance(alloc, mybir.MemoryLocationSet):
            continue
        name = alloc.memorylocations[0].name
        if alloc.kind == "ExternalInput":
            if name != partition_name:
                in_names.append(name)
        elif alloc.kind == "ExternalOutput":
            out_names.append(name)
            shape = tuple(alloc.tensor_shape)
            dtype = mybir.dt.np(alloc.dtype)
            out_avals.append(jax.core.ShapedArray(shape, dtype))
            zero_outs.append(np.zeros(shape, dtype))
    n_params = len(in_names)
    all_in_names = list(in_names) + list(out_names)
    if partition_name is not None:
        all_in_names.append(partition_name)

    def _body(*args):
        operands = list(args)
        if partition_name is not None:
            operands.append(bass2jax.partition_id_tensor())
        return tuple(bass2jax._bass_exec_p.bind(
            *operands,
            out_avals=tuple(out_avals),
            in_names=tuple(all_in_names),
            out_names=tuple(out_names),
            lowering_input_output_aliases=(),
            sim_require_finite=True,
            sim_require_nnan=True,
            nc=nc,
        ))

    devices = jax.devices()[:NCORES]
    mesh = Mesh(np.asarray(devices), ("core",))
    n_outs = len(out_names)
    in_specs = (PartitionSpec("core"),) * (n_params + n_outs)
    out_specs = (PartitionSpec("core"),) * n_outs
    f = jax.jit(shard_map(_body, mesh=mesh, in_specs=in_specs,
                          out_specs=out_specs, check_rep=False),
                keep_unused=True)
    sharding = NamedSharding(mesh, PartitionSpec("core"))
    dev_zero = [
        jax.device_put(np.concatenate([z] * NCORES, axis=0), sharding)
        for z in zero_outs
    ]
    _CACHE["runner"] = (f, sharding, in_names, out_names, dev_zero)
    return _CACHE["runner"]


def kernel(**inputs):
    import jax

    f, sharding, in_names, out_names, dev_zero = _runner()
    consts = {k: np.concatenate([v] * NCORES, axis=0)
              for k, v in _band_matrices_all().items()}
    dev_in = []
    for nm in in_names:
        arr = consts[nm] if nm in consts else np.ascontiguousarray(inputs[nm])
        dev_in.append(jax.device_put(arr, sharding))
    outs = f(*dev_in, *dev_zero)
    out = np.asarray(outs[out_names.index("out")])
    return out



# revision 38
# speedup vs baseline: 1.0163x; 1.0163x over previous
"""DeepFilter kernel for Trainium2 (8 NeuronCores, batch-parallel).

Math: the reference shifts input and filter by the SAME (df, dt) tap offset,
so the op factorizes into pointwise products followed by a separable 3x5
zero-padded box sum:
    P_r = ir*fr - ii*fi ; P_i = 2*ir*fi
    out_r = boxsum_3x5(P_r) ; out_i = boxsum_3x5(P_i)
    out = concat([out_r, out_i], axis=1)            # [B, 2F, T]

Per-core layout: F on partitions (2 full chunks + merged 12-row tail),
T on the free dim (TH-col pieces + 2-col halo).  The whole elementwise
chain runs on DVE as scalar_tensor_tensor ops (measured ~2x faster than
tensor_tensor, and keeping one engine means zero cross-engine semaphores
inside a piece); TensorE applies the F-box (banded matmul) accumulating
in PSUM; ScalarE copies PSUM->SBUF staging.

DMA queue layout (the big lever on this part): HBM->SBUF loads all go
through the SP HWDGE ring, which stripes a 128-partition transfer across
all 16 SDMA engines; SBUF->HBM stores go through the GPSIMD SWDGE ring.
NTFF profiles show HBM reads starve writes at the arbitration level:
store descriptors queue up but store bytes stop flowing whenever the
read queues are full, so execution alternates read phases (~410 GB/s)
and write phases (~165-210 GB/s) and the total is near their serialized
sum.  Stores on an HWDGE ring are far worse still (single-engine
~21 GB/s drain while reads are active, back-pressuring the pipeline),
so stores must live on the SWDGE ring.  Issuing loads from an engine
with no other work (SP) avoids head-of-line blocking -- an ACT-ring
load issue waiting on a tile-recycle semaphore stalls the PSUM-drain
copies queued behind it.
"""

import numpy as np

B, F, T = 16, 257, 4000
NCORES = 8
B_LOC = B // NCORES  # 2
P = 128
NT = 500  # psum tile width (<=512 fp32 matmul moving-operand limit)

# Regular F chunks: (first loaded row, n rows loaded,
#                    valid psum partitions [lo,hi), first output f row)
#  c0: rows 0..127   -> f 0..126  at partitions 0..126
#  c1: rows 126..253 -> f 127..252 at partitions 1..126
# The tail (f 253..256) is handled by a merged macro-tile covering BOTH
# batches: partitions b*6+r hold rows 251+r of batch b; a block-diagonal
# [12,8] band produces f 253..256 for b0 at partitions 0..3, b1 at 4..7.
CHUNKS = [
    (0, 128, 0, 127, 0),
    (126, 128, 1, 127, 127),
]
C2_FL0, C2_NROWS_B, C2_FO0, C2_NF = 251, 6, 253, 4

DEFAULT_TH = 1000
DEFAULT_BUFS = dict(inp=16, prod=8, pair=8, stg=8, ps=8)
# engine assignment: loads (ir, ii, fr, fi), stores (r, i)
DEFAULT_CFG = dict(load_eng=("sync", "sync", "sync", "sync"),
                   store_eng=("gpsimd", "gpsimd"), store_per_j=False,
                   elem="pe_i5", edge_split=False, const_eng="scalar")

_CACHE = {}


def _band_matrices():
    k = np.arange(P)
    band = (np.abs(k[:, None] - k[None, :]) <= 1).astype(np.float32)
    w6 = np.zeros((12, 8), np.float32)
    for bb in range(2):
        for r in range(6):
            for m in range(4):
                if abs(m + 2 - r) <= 1:
                    w6[bb * 6 + r, bb * 4 + m] = 1.0
    return band, w6


def _band_matrices_all():
    import ml_dtypes
    band, w6 = _band_matrices()
    return {"wp": band, "w6p": w6,
            "wpb": band.astype(ml_dtypes.bfloat16),
            "w6pb": w6.astype(ml_dtypes.bfloat16)}


def _build_module(repeats=1, th=DEFAULT_TH, bufs=None, dma_only=False,
                  cfg=None):
    import concourse.bacc as bacc
    import concourse.mybir as mybir
    import concourse.tile as tile

    cfg = dict(DEFAULT_CFG, **(cfg or {}))
    bufs = dict(DEFAULT_BUFS, **(bufs or {}))
    assert T % th == 0 and th % NT == 0
    n_pieces = T // th
    nj = th // NT
    tw = th + 4

    f32 = mybir.dt.float32
    f32r = mybir.dt.float32r
    mult = mybir.AluOpType.mult

    nc = bacc.Bacc("TRN2", target_bir_lowering=False, debug=False,
                   num_devices=NCORES)

    ins = {
        name: nc.dram_tensor(name, [B_LOC, F, T], f32, kind="ExternalInput")
        for name in ("inputs_r", "inputs_i", "filters_r", "filters_i")
    }
    bf16 = mybir.dt.bfloat16
    wp_d = nc.dram_tensor("wp", [P, P], f32r, kind="ExternalInput")
    w6p_d = nc.dram_tensor("w6p", [12, 8], f32r, kind="ExternalInput")
    wpb_d = nc.dram_tensor("wpb", [P, P], bf16, kind="ExternalInput")
    w6pb_d = nc.dram_tensor("w6pb", [12, 8], bf16, kind="ExternalInput")
    out_d = nc.dram_tensor("out", [B_LOC, 2 * F, T], f32, kind="ExternalOutput")

    ir_ap, ii_ap, fr_ap, fi_ap = (ins[n].ap() for n in
                                  ("inputs_r", "inputs_i", "filters_r",
                                   "filters_i"))
    out_ap = out_d.ap()

    with tile.TileContext(nc) as tc:
        with (
            tc.tile_pool(name="const", bufs=1) as cpool,
            tc.tile_pool(name="inp", bufs=bufs["inp"]) as ipool,
            tc.tile_pool(name="prod", bufs=bufs["prod"]) as rpool,
            tc.tile_pool(name="pair", bufs=bufs["pair"]) as wpool,
            tc.tile_pool(name="stg", bufs=bufs["stg"]) as spool,
            tc.tile_pool(name="ps", bufs=bufs["ps"], space="PSUM") as qpool,
        ):
            wp_s = cpool.tile([P, P], f32r, name="wp_s", tag="wp_s")
            w6p_s = cpool.tile([12, 8], f32r, name="w6p_s", tag="w6p_s")
            wpb_s = cpool.tile([P, P], bf16, name="wpb_s", tag="wpb_s")
            w6pb_s = cpool.tile([12, 8], bf16, name="w6pb_s", tag="w6pb_s")
            # const loads go out the ACT ring (idle at t=0) so they don't
            # sit ahead of piece-0's input loads in the SP ring
            ce = getattr(nc, cfg.get("const_eng", "sync"))
            ce.dma_start(out=wp_s[:, :], in_=wp_d.ap()[:, :])
            ce.dma_start(out=w6p_s[:, :], in_=w6p_d.ap()[:, :])
            ce.dma_start(out=wpb_s[:, :], in_=wpb_d.ap()[:, :])
            ce.dma_start(out=w6pb_s[:, :], in_=w6pb_d.ap()[:, :])

            def emit_piece(t0, thp, loads, nrows, wpL, wpLb, np_out, vp1,
                           stores, stg_pair=None):
                """One macro-tile: T cols [t0, t0+thp), given per-batch
                loads [(part_off, b, fl0, nr)], band slices, valid psum
                rows [0,vp1), stores [(stage p0, p1, b, first f row)].
                stg_pair=(stg_r, stg_i, col_off, flush, store_t0,
                store_w): write staging at col_off and only emit the
                (wider) store DMA when flush -- wider store rows win
                SDMA arbitration slots against concurrent reads."""
                tw_p = thp + 4
                nj_p = thp // NT
                # tile col c <-> t = t0 - 2 + c ; clip to [0, T)
                c_lo = max(0, 2 - t0)
                c_hi = tw_p - max(0, t0 + thp + 2 - T)
                t_lo, t_hi = t0 - 2 + c_lo, t0 - 2 + c_hi

                ir_t = ipool.tile([P, tw_p], f32, name="ir_t", tag="inp")
                ii_t = ipool.tile([P, tw_p], f32, name="ii_t", tag="inp")
                fr_t = ipool.tile([P, tw_p], f32, name="fr_t", tag="inp")
                fi_t = ipool.tile([P, tw_p], f32, name="fi_t", tag="inp")
                # loads split across DMA issue paths per cfg so transfers
                # from different tensors can overlap instead of sitting
                # FIFO in one ring
                le = [getattr(nc, e) for e in cfg["load_eng"]]
                # ir, fr first: the chain's first op t1 = ir*fr can
                # start as soon as two transfers land
                for t_sb, src, eng in ((ir_t, ir_ap, le[0]),
                                       (fr_t, fr_ap, le[2]),
                                       (ii_t, ii_ap, le[1]),
                                       (fi_t, fi_ap, le[3])):
                    for p_off, b, fl0, nr in loads:
                        eng.dma_start(
                            out=t_sb[p_off:p_off + nr, c_lo:c_hi],
                            in_=src[b, fl0:fl0 + nr, t_lo:t_hi])
                    # zero halo cols at the global T edges so the products
                    # are zero there (zero-pad semantics) and matmuls can
                    # always run full-width (fp32r needs even widths)
                    if c_lo > 0:
                        nc.vector.memset(t_sb[0:nrows, 0:c_lo], 0.0)
                    if c_hi < tw_p:
                        nc.vector.memset(t_sb[0:nrows, c_hi:tw_p], 0.0)

                if dma_only:
                    # measurement variant: identical DMA traffic, no
                    # compute -- stores forward slices of the loads
                    de_r, de_i = (getattr(nc, e) for e in cfg["store_eng"])
                    for sp0, sp1, b, fo0 in stores:
                        n_f = sp1 - sp0
                        de_r.dma_start(
                            out=out_ap[b, fo0:fo0 + n_f, t0:t0 + thp],
                            in_=ir_t[sp0:sp1, 2:2 + thp])
                        de_i.dma_start(
                            out=out_ap[b, F + fo0:F + fo0 + n_f, t0:t0 + thp],
                            in_=ii_t[sp0:sp1, 2:2 + thp])
                    return

                elem = cfg["elem"]
                add = mybir.AluOpType.add
                sub = mybir.AluOpType.subtract
                nr = nrows

                def stt(out, a, scl, b, op1):
                    # scalar_tensor_tensor runs ~2x faster than
                    # tensor_tensor on DVE (measured): express every
                    # elementwise op as (in0 op0 scalar) op1 in1
                    nc.vector.scalar_tensor_tensor(
                        out=out, in0=a, scalar=scl, in1=b,
                        op0=mult, op1=op1)

                if elem == "pe_rij5":
                    # 3 DVE products in bf16; the whole 5-tap T-box and
                    # the r-plane subtraction run on PE: 15 accumulating
                    # matmuls per psum pair, all sharing one bf16 band
                    # (the minus is folded into t2 = -(ii*fi)).
                    t1_t = rpool.tile([P, tw_p], bf16, name="t1_t",
                                      tag="prod")
                    t2_t = rpool.tile([P, tw_p], bf16, name="t2_t",
                                      tag="prod")
                    pi_t = rpool.tile([P, tw_p], bf16, name="pi_t",
                                      tag="prod")
                    stt(t1_t[0:nr, 0:tw_p], ir_t[0:nr, 0:tw_p],
                        1.0, fr_t[0:nr, 0:tw_p], mult)
                    stt(t2_t[0:nr, 0:tw_p], ii_t[0:nr, 0:tw_p],
                        -1.0, fi_t[0:nr, 0:tw_p], mult)
                    stt(pi_t[0:nr, 0:tw_p], ir_t[0:nr, 0:tw_p],
                        2.0, fi_t[0:nr, 0:tw_p], mult)

                    def groups_for(j, ps_r, ps_i):
                        return (
                            (ps_i,
                             [(pi_t, wpLb, NT * j + s) for s in range(5)]),
                            (ps_r,
                             [(t1_t, wpLb, NT * j + s) for s in range(5)]
                             + [(t2_t, wpLb, NT * j + s)
                                for s in range(5)]),
                        )
                elif elem == "pe_i5":
                    # i-plane skips its pair sum: 5 direct bf16 matmuls
                    # on pi; r-plane keeps the fp32r pair-sum scheme
                    t1_t = rpool.tile([P, tw_p], f32r, name="t1_t",
                                      tag="prod")
                    t2_t = rpool.tile([P, tw_p], f32r, name="t2_t",
                                      tag="prod")
                    pi_t = rpool.tile([P, tw_p], bf16, name="pi_t",
                                      tag="prod")
                    pr_t = t1_t
                    qr_t = wpool.tile([P, tw_p], f32r, name="qr_t",
                                      tag="pair")
                    stt(t1_t[0:nr, 0:tw_p], ir_t[0:nr, 0:tw_p],
                        1.0, fr_t[0:nr, 0:tw_p], mult)
                    stt(t2_t[0:nr, 0:tw_p], ii_t[0:nr, 0:tw_p],
                        1.0, fi_t[0:nr, 0:tw_p], mult)
                    stt(pi_t[0:nr, 0:tw_p], ir_t[0:nr, 0:tw_p],
                        2.0, fi_t[0:nr, 0:tw_p], mult)
                    stt(pr_t[0:nr, 0:tw_p], t1_t[0:nr, 0:tw_p],
                        1.0, t2_t[0:nr, 0:tw_p], sub)
                    stt(qr_t[0:nr, 0:tw_p - 1],
                        pr_t[0:nr, 0:tw_p - 1], 1.0, pr_t[0:nr, 1:tw_p], add)

                    def groups_for(j, ps_r, ps_i):
                        return (
                            (ps_i,
                             [(pi_t, wpLb, NT * j + s) for s in range(5)]),
                            (ps_r,
                             [(qr_t, wpL, NT * j),
                              (qr_t, wpL, NT * j + 2),
                              (pr_t, wpL, NT * j + 4)]),
                        )
                else:
                    # float32r: PE matmuls on fp32r run 4x faster than
                    # fp32; DVE rounds the products on write.  pr = t1-t2
                    # on DVE means both planes share ONE wp band: 6
                    # matmuls per psum pair.
                    t1_t = rpool.tile([P, tw_p], f32r, name="t1_t",
                                      tag="prod")
                    t2_t = rpool.tile([P, tw_p], f32r, name="t2_t",
                                      tag="prod")
                    pi_t = rpool.tile([P, tw_p], f32r, name="pi_t",
                                      tag="prod")
                    pr_t = t1_t
                    qr_t = wpool.tile([P, tw_p], f32r, name="qr_t",
                                      tag="pair")
                    qi_t = wpool.tile([P, tw_p], f32r, name="qi_t",
                                      tag="pair")
                    if elem.startswith("stt"):
                        stt(t1_t[0:nr, 0:tw_p], ir_t[0:nr, 0:tw_p],
                            1.0, fr_t[0:nr, 0:tw_p], mult)
                        stt(t2_t[0:nr, 0:tw_p], ii_t[0:nr, 0:tw_p],
                            1.0, fi_t[0:nr, 0:tw_p], mult)
                        stt(pi_t[0:nr, 0:tw_p], ir_t[0:nr, 0:tw_p],
                            2.0, fi_t[0:nr, 0:tw_p], mult)
                        # gpsimd cannot run STT (TensorScalarPtr is not
                        # a Pool-engine op on V3) -- keep the chain on
                        # DVE: in-order issue, zero cross-engine sems
                        stt(pr_t[0:nr, 0:tw_p], t1_t[0:nr, 0:tw_p],
                            1.0, t2_t[0:nr, 0:tw_p], sub)
                        if elem == "stt_qr_gps":
                            nc.gpsimd.tensor_add(qr_t[0:nr, 0:tw_p - 1],
                                                 pr_t[0:nr, 0:tw_p - 1],
                                                 pr_t[0:nr, 1:tw_p])
                        else:
                            stt(qr_t[0:nr, 0:tw_p - 1],
                                pr_t[0:nr, 0:tw_p - 1], 1.0,
                                pr_t[0:nr, 1:tw_p], add)
                        stt(qi_t[0:nr, 0:tw_p - 1],
                            pi_t[0:nr, 0:tw_p - 1], 1.0, pi_t[0:nr, 1:tw_p],
                            add)
                    else:
                        nc.vector.tensor_mul(t1_t[0:nr, 0:tw_p],
                                             ir_t[0:nr, 0:tw_p],
                                             fr_t[0:nr, 0:tw_p])
                        nc.vector.tensor_mul(t2_t[0:nr, 0:tw_p],
                                             ii_t[0:nr, 0:tw_p],
                                             fi_t[0:nr, 0:tw_p])
                        nc.vector.scalar_tensor_tensor(
                            out=pi_t[0:nr, 0:tw_p],
                            in0=ir_t[0:nr, 0:tw_p], scalar=2.0,
                            in1=fi_t[0:nr, 0:tw_p], op0=mult, op1=mult)
                        nc.gpsimd.tensor_sub(pr_t[0:nr, 0:tw_p],
                                             t1_t[0:nr, 0:tw_p],
                                             t2_t[0:nr, 0:tw_p])
                        nc.gpsimd.tensor_add(qr_t[0:nr, 0:tw_p - 1],
                                             pr_t[0:nr, 0:tw_p - 1],
                                             pr_t[0:nr, 1:tw_p])
                        nc.vector.tensor_add(qi_t[0:nr, 0:tw_p - 1],
                                             pi_t[0:nr, 0:tw_p - 1],
                                             pi_t[0:nr, 1:tw_p])

                    def groups_for(j, ps_r, ps_i):
                        return (
                            (ps_i,
                             [(qi_t, wpL, NT * j),
                              (qi_t, wpL, NT * j + 2),
                              (pi_t, wpL, NT * j + 4)]),
                            (ps_r,
                             [(qr_t, wpL, NT * j),
                              (qr_t, wpL, NT * j + 2),
                              (pr_t, wpL, NT * j + 4)]),
                        )

                if stg_pair is None:
                    stg_r = spool.tile([P, thp], f32, name="stg_r",
                                       tag="stg")
                    stg_i = spool.tile([P, thp], f32, name="stg_i",
                                       tag="stg")
                    col_off, flush, store_t0, store_w = 0, True, t0, thp
                else:
                    stg_r, stg_i, col_off, flush, store_t0, store_w = \
                        stg_pair
                se_r, se_i = (getattr(nc, e) for e in cfg["store_eng"])

                for j in range(nj_p):
                    ps_r = qpool.tile([P, NT], f32, name="ps_r", tag="ps")
                    ps_i = qpool.tile([P, NT], f32, name="ps_i", tag="ps")
                    for ps, mms in groups_for(j, ps_r, ps_i):
                        for k, (plane, wL, c_start) in enumerate(mms):
                            nc.tensor.matmul(
                                ps[0:np_out, 0:NT],
                                wL,
                                plane[0:nrows, c_start:c_start + NT],
                                start=(k == 0),
                                stop=(k == len(mms) - 1))
                    # PSUM reads must start at partition 0: copy rows
                    # 0:vp1 and let the store DMAs pick their slices.
                    nc.scalar.copy(
                        out=stg_r[0:vp1,
                                  col_off + NT * j:col_off + NT * (j + 1)],
                        in_=ps_r[0:vp1, 0:NT])
                    nc.scalar.copy(
                        out=stg_i[0:vp1,
                                  col_off + NT * j:col_off + NT * (j + 1)],
                        in_=ps_i[0:vp1, 0:NT])
                    if cfg["store_per_j"]:
                        for sp0, sp1, b, fo0 in stores:
                            n_f = sp1 - sp0
                            t_j0 = t0 + NT * j
                            se_r.dma_start(
                                out=out_ap[b, fo0:fo0 + n_f,
                                           t_j0:t_j0 + NT],
                                in_=stg_r[sp0:sp1, NT * j:NT * (j + 1)])
                            se_i.dma_start(
                                out=out_ap[b, F + fo0:F + fo0 + n_f,
                                           t_j0:t_j0 + NT],
                                in_=stg_i[sp0:sp1, NT * j:NT * (j + 1)])

                if not cfg["store_per_j"] and flush:
                    for sp0, sp1, b, fo0 in stores:
                        n_f = sp1 - sp0
                        se_r.dma_start(
                            out=out_ap[b, fo0:fo0 + n_f,
                                       store_t0:store_t0 + store_w],
                            in_=stg_r[sp0:sp1, 0:store_w])
                        se_i.dma_start(
                            out=out_ap[b, F + fo0:F + fo0 + n_f,
                                       store_t0:store_t0 + store_w],
                            in_=stg_i[sp0:sp1, 0:store_w])

            def chunk_schedule(split_head, split_tail):
                """(t0, thp) pieces covering [0, T); optionally split the
                first/last th-piece into NT-wide slivers to shorten the
                stream's pipeline fill/drain latency."""
                s = [(th * h, th) for h in range(n_pieces)]
                out = []
                for i, (t0, thp) in enumerate(s):
                    if (split_head and i == 0) or (split_tail and
                                                   i == n_pieces - 1):
                        out += [(t0 + k * NT, NT) for k in range(thp // NT)]
                    else:
                        out.append((t0, thp))
                return out

            # piece sequence: regular (b, chunk, h) pieces, with the
            # DMA-light merged-tail pieces interleaved mid-stream
            edge = cfg.get("edge_split", False)
            for _rep in range(repeats):
                half = (n_pieces + 1) // 2
                n_ch = len(CHUNKS)
                pair_n = cfg.get("store_pair", 1)
                for b in range(B_LOC):
                    for ci, (fl0, nrows, vp0, vp1, fo0) in enumerate(CHUNKS):
                        first = edge and b == 0 and ci == 0
                        last = edge and b == B_LOC - 1 and ci == n_ch - 1
                        sched = chunk_schedule(first, last)
                        carry = None
                        for pi_, (t0, thp) in enumerate(sched):
                            sp = None
                            if pair_n > 1 and thp == th:
                                k = pi_ % pair_n
                                if k == 0:
                                    carry = (
                                        spool.tile([P, pair_n * th], f32,
                                                   name="stg_r", tag="stg"),
                                        spool.tile([P, pair_n * th], f32,
                                                   name="stg_i", tag="stg"),
                                        t0)
                                sp = (carry[0], carry[1], k * th,
                                      k == pair_n - 1, carry[2],
                                      pair_n * th)
                            emit_piece(
                                t0, thp, [(0, b, fl0, nrows)], nrows,
                                wp_s[:, :], wpb_s[:, :], P, vp1,
                                [(vp0, vp1, b, fo0)], stg_pair=sp)
                    # merged tail pieces: first half after batch 0,
                    # second half after batch 1
                    hs = range(0, half) if b == 0 else range(half, n_pieces)
                    for h in hs:
                        emit_piece(
                            th * h, th,
                            [(0, 0, C2_FL0, C2_NROWS_B),
                             (6, 1, C2_FL0, C2_NROWS_B)],
                            12, w6p_s[0:12, 0:8], w6pb_s[0:12, 0:8], 8, 8,
                            [(0, 4, 0, C2_FO0), (4, 8, 1, C2_FO0)])

    nc.compile()
    return nc


def _get_module(repeats=1, th=DEFAULT_TH, bufs=None, dma_only=False,
                cfg=None):
    key = (f"nc{repeats}_{th}_{sorted((bufs or {}).items())}_{dma_only}"
           f"_{sorted((cfg or {}).items())}")
    if key not in _CACHE:
        _CACHE[key] = _build_module(repeats, th, bufs, dma_only, cfg)
    return _CACHE[key]


def _runner():
    """Build (once) a reusable jitted 8-core runner for the module."""
    if "runner" in _CACHE:
        return _CACHE["runner"]
    import jax
    import concourse.mybir as mybir
    from concourse import bass2jax
    from jax.sharding import Mesh, NamedSharding, PartitionSpec
    from jax.experimental.shard_map import shard_map

    nc = _get_module()
    bass2jax.install_neuronx_cc_hook()

    partition_name = (nc.partition_id_tensor.name
                      if nc.partition_id_tensor else None)
    in_names, out_names, out_avals, zero_outs = [], [], [], []
    for alloc in nc.m.functions[0].allocations:
        if not isinstance(alloc, mybir.MemoryLocationSet):
            continue
        name = alloc.memorylocations[0].name
        if alloc.kind == "ExternalInput":
            if name != partition_name:
                in_names.append(name)
        elif alloc.kind == "ExternalOutput":
            out_names.append(name)
            shape = tuple(alloc.tensor_shape)
            dtype = mybir.dt.np(alloc.dtype)
            out_avals.append(jax.core.ShapedArray(shape, dtype))
            zero_outs.append(np.zeros(shape, dtype))
    n_params = len(in_names)
    all_in_names = list(in_names) + list(out_names)
    if partition_name is not None:
        all_in_names.append(partition_name)

    def _body(*args):
        operands = list(args)
        if partition_name is not None:
            operands.append(bass2jax.partition_id_tensor())
        return tuple(bass2jax._bass_exec_p.bind(
            *operands,
            out_avals=tuple(out_avals),
            in_names=tuple(all_in_names),
            out_names=tuple(out_names),
            lowering_input_output_aliases=(),
            sim_require_finite=True,
            sim_require_nnan=True,
            nc=nc,
        ))

    devices = jax.devices()[:NCORES]
    mesh = Mesh(np.asarray(devices), ("core",))
    n_outs = len(out_names)
    in_specs = (PartitionSpec("core"),) * (n_params + n_outs)
    out_specs = (PartitionSpec("core"),) * n_outs
    f = jax.jit(shard_map(_body, mesh=mesh, in_specs=in_specs,
                          out_specs=out_specs, check_rep=False),
                keep_unused=True)
    sharding = NamedSharding(mesh, PartitionSpec("core"))
    dev_zero = [
        jax.device_put(np.concatenate([z] * NCORES, axis=0), sharding)
        for z in zero_outs
    ]
    _CACHE["runner"] = (f, sharding, in_names, out_names, dev_zero)
    return _CACHE["runner"]


def kernel(**inputs):
    import jax

    f, sharding, in_names, out_names, dev_zero = _runner()
    consts = {k: np.concatenate([v] * NCORES, axis=0)
              for k, v in _band_matrices_all().items()}
    dev_in = []
    for nm in in_names:
        arr = consts[nm] if nm in consts else np.ascontiguousarray(inputs[nm])
        dev_in.append(jax.device_put(arr, sharding))
    outs = f(*dev_in, *dev_zero)
    out = np.asarray(outs[out_names.index("out")])
    return out

